# revision 8
# baseline (speedup 1.0000x reference)
"""Trainium2 Bass kernel for nn_DendriticBranchLayer.

rate = alpha * relu(V - Vth)^2,  V = (exc + cur) / (exc + 1 + cond + inh)
  exc = x @ pruned(pre_w_exc, K=32).T        [B, OUT]
  inh = inhibitory_input @ pruned(pre_w_inh, K=16).T
  cur = sum_f branch_input.reshape(B,OUT,4)[...,f] * w_block[:,f]

Strategy: the top-K masked weights depend only on the small weight tensors, so
the dense masked weights are materialized on the host. The matmuls dominate
(dense 6144x1024 contraction per batch row), so they run in fp8-e4m3 with
perf_mode=DoubleRow: two fp8 weights per PE cell -> a 256-deep contraction per
instruction at ~2x fp16 throughput, free dim 512 (one full PSUM bank) so the
per-matmul LDWEIGHTS hides under the 512-column stream. Weights are pre-scaled
by S; the scale cancels in V = num/den because br is pre-scaled by S on host
(fp16, no precision loss) and the (1+cond) constant is stored as S*(1+cond).
End-to-end rel-l2 error vs the fp64 reference is ~9.6e-3 (fp8 quantization of
x and the weights).

Batch dim is sharded over 8 cores. On each core: outputs live on PSUM
partitions (128 outputs/block), batch on the free dim, so all per-output
constants are per-partition scalars fed straight into fused DVE/ACT ops.

The kernel start is DMA-ramp-bound (the PE consumes rhs ~2x faster than HBM
delivers, and the SDMA engines round-robin across all outstanding transfers,
so everything queued dilutes the critical first tiles). Block 0's operands
are therefore split into arrival-sized pieces and its matmuls are emitted in
arrival order, interleaving the exc/inh accumulation groups (separate PSUM
banks); bulk prefetches (br, next weights, constants) are issued behind them.

Pointwise tail per block: cur = strided DVE reduce over the f-minor br tile
(fp16 in AND out -> DVE 2x_1P packs two 16-bit lanes), num = exc+cur (DVE),
exc1 = exc + S(1+cond) (ACT, a DVE op may read only one PSUM operand),
den = exc1+inh (DVE), rden (DVE fast reciprocal), v = num*rden (GpSimd),
relu-shift and scaled-square (ACT) -> fp16 output DMA on the ACT HWDGE ring
(keeps waiting output DMAs out of the input-load FIFO).
"""

import numpy as np
import ml_dtypes

import concourse.bass as bass
import concourse.mybir as mybir
import concourse.tile as tile
from concourse import bacc
from concourse.bass_utils import run_bass_kernel_spmd

B, OUT, EXC_IN, INH_IN, BF = 8192, 1024, 4096, 2048, 4
K_EXC, K_INH = 32, 16

NCORES = 8
BC = B // NCORES          # batch per core (1024)
P = 128                   # partitions
NB = 2                    # batch sub-blocks per core
BSUB = BC // NB           # 512 batch per sub-block (one PSUM bank of fp32)
OB = OUT // P             # 8 output blocks
KE = EXC_IN // P          # 32 contraction chunks (exc)
KI = INH_IN // P          # 16 contraction chunks (inh)
JE = KE // 2              # 16 DoubleRow matmuls (exc)
JI = KI // 2              # 8 DoubleRow matmuls (inh)
KQ = 8                    # k-chunks in the first xt subtile
KQI = 4                   # k-chunks in the first iht subtile

S = 13.8                  # weight/br/current scale (cancels in num/den)

F8 = ml_dtypes.float8_e4m3

# cst column layout: [P, 3*OB + OB*BF]
_C_CP1 = 0                # S * (1 + cond), per output
_C_VTHN = OB              # -Vth, per output
_C_SA = 2 * OB            # sqrt(alpha), per output
_C_WB = 3 * OB            # w_block[o, ob*BF + f]
_C_COLS = 3 * OB + OB * BF

_CACHE = {}
TRACE = False  # set by test harness to capture an NTFF profile


def _build_program(wb_ones):
    nc = bacc.Bacc("TRN2", target_bir_lowering=False, debug=False)
    f8, f16, f32 = mybir.dt.float8e4, mybir.dt.float16, mybir.dt.float32
    DR = mybir.MatmulPerfMode.DoubleRow

    wte = nc.declare_dram_parameter("wte", [P, OB, KE, P], f8, isOutput=False)
    wti = nc.declare_dram_parameter("wti", [P, OB, KI, P], f8, isOutput=False)
    xt = nc.declare_dram_parameter("xt", [NB, P, KE, BSUB], f8, isOutput=False)
    iht = nc.declare_dram_parameter("iht", [NB, P, KI, BSUB], f8, isOutput=False)
    brt = nc.declare_dram_parameter("brt", [NB, OB, P, BSUB, BF], f16, isOutput=False)
    cst = nc.declare_dram_parameter("cst", [P, _C_COLS], f32, isOutput=False)
    outt = nc.declare_dram_parameter("outt", [OB, P, NB, BSUB], f16, isOutput=True)

    add = mybir.AluOpType.add
    mult = mybir.AluOpType.mult
    AxX = mybir.AxisListType.X
    Relu = mybir.ActivationFunctionType.Relu
    Square = mybir.ActivationFunctionType.Square
    Identity = mybir.ActivationFunctionType.Identity

    def pieces_slice(pieces, j):
        """Find the piece covering k-chunks [2j, 2j+2) -> sliced 3D AP."""
        k = 2 * j
        for t, base, cnt in pieces:
            if base <= k and k + 2 <= base + cnt:
                return t[:, k - base:k - base + 2, :]
        raise AssertionError(f"no piece for k={k}")

    with tile.TileContext(nc) as tc:
        with tc.tile_pool(name="wpool", bufs=1) as wpool, \
             tc.tile_pool(name="xpool", bufs=2) as xpool, \
             tc.tile_pool(name="ipool", bufs=2) as ipool, \
             tc.tile_pool(name="brpool", bufs=4) as brpool, \
             tc.tile_pool(name="wk", bufs=3) as wk, \
             tc.tile_pool(name="wk2", bufs=1) as wk2, \
             tc.tile_pool(name="ps_exc", bufs=4, space="PSUM") as ps_exc, \
             tc.tile_pool(name="ps_inh", bufs=4, space="PSUM") as ps_inh:

            # weights per ob: list of (tile, k_base, k_cnt)
            wte_sb, wti_sb = [None] * OB, [None] * OB

            def load_weights(ob):
                if ob >= OB or wte_sb[ob] is not None:
                    return
                we = wpool.tile([P, KE, P], f8, tag=f"wte{ob}")
                nc.sync.dma_start(out=we, in_=wte[:, ob, :, :])
                wte_sb[ob] = [(we, 0, KE)]
                wi = wpool.tile([P, KI, P], f8, tag=f"wti{ob}")
                nc.sync.dma_start(out=wi, in_=wti[:, ob, :, :])
                wti_sb[ob] = [(wi, 0, KI)]

            # ---- block-0 critical lead-in, in DMA-arrival order ----------
            # (wti0a, iha) feed the first two inh matmuls; (wte0a, xta) the
            # first four exc matmuls; the large remainders follow.
            wi0a = wpool.tile([P, KQI, P], f8, tag="wti0a")
            nc.sync.dma_start(out=wi0a, in_=wti[:, 0, 0:KQI, :])
            ih0a = ipool.tile([P, KQI, BSUB], f8, tag="ihta")
            nc.sync.dma_start(out=ih0a, in_=iht[0, :, 0:KQI, :])
            we0a = wpool.tile([P, KQ, P], f8, tag="wte0a")
            nc.sync.dma_start(out=we0a, in_=wte[:, 0, 0:KQ, :])
            xs0a = xpool.tile([P, KQ, BSUB], f8, tag="xta")
            nc.sync.dma_start(out=xs0a, in_=xt[0, :, 0:KQ, :])
            wi0b = wpool.tile([P, KI - KQI, P], f8, tag="wti0b")
            nc.sync.dma_start(out=wi0b, in_=wti[:, 0, KQI:KI, :])
            ih0b = ipool.tile([P, KI - KQI, BSUB], f8, tag="ihtb")
            nc.sync.dma_start(out=ih0b, in_=iht[0, :, KQI:KI, :])
            we0b = wpool.tile([P, KE - KQ, P], f8, tag="wte0b")
            nc.sync.dma_start(out=we0b, in_=wte[:, 0, KQ:KE, :])
            xs0b = xpool.tile([P, KE - KQ, BSUB], f8, tag="xtb")
            nc.sync.dma_start(out=xs0b, in_=xt[0, :, KQ:KE, :])
            wti_sb[0] = [(wi0a, 0, KQI), (wi0b, KQI, KI - KQI)]
            wte_sb[0] = [(we0a, 0, KQ), (we0b, KQ, KE - KQ)]
            xi_tiles = {0: ([(xs0a, 0, KQ), (xs0b, KQ, KE - KQ)],
                            [(ih0a, 0, KQI), (ih0b, KQI, KI - KQI)])}
            cst_s = wpool.tile([P, _C_COLS], f32)
            nc.sync.dma_start(out=cst_s, in_=cst[:, :])

            def load_nb(nb):
                if nb >= NB or nb in xi_tiles:
                    return
                iha = ipool.tile([P, KQI, BSUB], f8, tag="ihta")
                nc.sync.dma_start(out=iha, in_=iht[nb, :, 0:KQI, :])
                xsa = xpool.tile([P, KQ, BSUB], f8, tag="xta")
                nc.sync.dma_start(out=xsa, in_=xt[nb, :, 0:KQ, :])
                ihb = ipool.tile([P, KI - KQI, BSUB], f8, tag="ihtb")
                nc.sync.dma_start(out=ihb, in_=iht[nb, :, KQI:KI, :])
                xsb = xpool.tile([P, KE - KQ, BSUB], f8, tag="xtb")
                nc.sync.dma_start(out=xsb, in_=xt[nb, :, KQ:KE, :])
                xi_tiles[nb] = ([(xsa, 0, KQ), (xsb, KQ, KE - KQ)],
                                [(iha, 0, KQI), (ihb, KQI, KI - KQI)])

            for nb in range(NB):
                x_pieces, ih_pieces = xi_tiles[nb]

                for ob in range(OB):
                    first = nb == 0 and ob == 0
                    br_s = None
                    if not first:
                        br_s = brpool.tile([P, BSUB, BF], f16, tag="br")
                        nc.sync.dma_start(out=br_s, in_=brt[nb, ob, :, :, :])
                        if nb == 0:
                            load_weights(ob + 1)
                            if ob == 1:
                                load_weights(ob + 2)
                        if nb == 0 and ob == 3:
                            load_nb(1)

                    exc_ps = ps_exc.tile([P, BSUB], f32, tag="exc")
                    inh_ps = ps_inh.tile([P, BSUB], f32, tag="inh")

                    def emit_inh(j0, j1):
                        for j in range(j0, j1):
                            nc.tensor.matmul(
                                inh_ps, pieces_slice(wti_sb[ob], j),
                                pieces_slice(ih_pieces, j),
                                start=(j == 0), stop=(j == JI - 1),
                                perf_mode=DR)

                    def emit_exc(j0, j1):
                        for j in range(j0, j1):
                            nc.tensor.matmul(
                                exc_ps, pieces_slice(wte_sb[ob], j),
                                pieces_slice(x_pieces, j),
                                start=(j == 0), stop=(j == JE - 1),
                                perf_mode=DR)

                    if first:
                        # arrival-ordered, groups interleaved across banks
                        emit_inh(0, KQI // 2)
                        emit_exc(0, KQ // 2)
                        emit_inh(KQI // 2, JI)
                        emit_exc(KQ // 2, JE)
                        # non-critical DMAs enter the rings after the above
                        br_s = brpool.tile([P, BSUB, BF], f16, tag="br")
                        nc.sync.dma_start(out=br_s, in_=brt[nb, ob, :, :, :])
                        load_weights(1)
                    else:
                        emit_exc(0, JE)
                        emit_inh(0, JI)

                    def pointwise(pool, c0, w, sfx):
                        cs = slice(c0, c0 + w)
                        # cur' = S*cur (br pre-scaled by S on host); fp16 in
                        # and out so the DVE runs the reduce in 2x_1P mode
                        if wb_ones:
                            cur = pool.tile([P, w], f16, tag="cur" + sfx)
                            # 4-term sum of O(10) values: fp16 out is exact to
                            # ~5e-4 and enables the DVE 2x_1P packed mode
                            with nc.allow_low_precision(reason="cur is a 4-term fp16-range sum"):
                                nc.vector.tensor_reduce(
                                    cur, br_s[:, cs, :], axis=AxX, op=add)
                        else:
                            cur = pool.tile([P, w], f32, tag="curg" + sfx)
                            nc.gpsimd.tensor_scalar_mul(
                                cur, br_s[:, cs, 0],
                                cst_s[:, _C_WB + ob * BF: _C_WB + ob * BF + 1])
                            for f in range(1, BF):
                                nxt = pool.tile([P, w], f32, tag=f"cur{f % 2}" + sfx)
                                nc.gpsimd.scalar_tensor_tensor(
                                    nxt, br_s[:, cs, f],
                                    cst_s[:, _C_WB + ob * BF + f: _C_WB + ob * BF + f + 1],
                                    cur, op0=mult, op1=add)
                                cur = nxt

                        num = pool.tile([P, w], f32, tag="num" + sfx)
                        nc.vector.tensor_add(num, exc_ps[:, cs], cur)
                        # exc1 = exc' + S*(1+cond) on ACT (a DVE op may read
                        # only one PSUM operand, so den takes two ops)
                        exc1 = pool.tile([P, w], f32, tag="exc1" + sfx)
                        nc.scalar.activation(
                            exc1, exc_ps[:, cs], Identity,
                            bias=cst_s[:, _C_CP1 + ob: _C_CP1 + ob + 1])
                        den = pool.tile([P, w], f32, tag="den" + sfx)
                        nc.vector.tensor_add(den, exc1, inh_ps[:, cs])
                        rden = pool.tile([P, w], f32, tag="rden" + sfx)
                        nc.vector.reciprocal_approx_fast(rden, den)
                        v = pool.tile([P, w], f32, tag="v" + sfx)
                        nc.gpsimd.tensor_mul(v, num, rden)
                        # r = relu(v - Vth); rate = (r * sqrt(alpha))^2
                        r = pool.tile([P, w], f32, tag="r" + sfx)
                        nc.scalar.activation(
                            r, v, Relu, bias=cst_s[:, _C_VTHN + ob: _C_VTHN + ob + 1])
                        ot = pool.tile([P, w], f16, tag="ot" + sfx)
                        nc.scalar.activation(
                            ot, r, Square, scale=cst_s[:, _C_SA + ob: _C_SA + ob + 1])
                        # out DMA on the ACT HWDGE ring: follows the ot
                        # activation on the same queue
                        nc.scalar.dma_start(out=outt[ob, :, nb, cs], in_=ot)

                    if nb == NB - 1 and ob == OB - 1:
                        # split the final chain so the kernel tail is shorter
                        pointwise(wk2, 0, BSUB // 2, "h0")
                        pointwise(wk2, BSUB // 2, BSUB // 2, "h1")
                    else:
                        pointwise(wk, 0, BSUB, "")

    nc.compile()
    return nc


def _pruned_dense_T(pre_w, K):
    """Masked weight, transposed to [in, out] fp32. Tie-break matches
    jax.lax.top_k: equal values -> lower index wins (stable sort)."""
    idx = np.argsort(-pre_w, axis=1, kind="stable")[:, :K]
    w = np.exp(pre_w.astype(np.float32))
    dense = np.zeros(pre_w.shape, dtype=np.float32)
    np.put_along_axis(dense, idx, np.take_along_axis(w, idx, axis=1), axis=1)
    return dense.T


def kernel(x, inhibitory_input, branch_input, pre_w_exc, pre_w_inh,
           w_block, presigmoid_Vth, log_alpha_max):
    w_block = np.asarray(w_block, dtype=np.float32)
    wb_ones = bool(np.all(w_block == 1.0))
    key = ("nc", wb_ones)
    if key not in _CACHE:
        _CACHE[key] = _build_program(wb_ones)
    nc = _CACHE[key]

    x = np.ascontiguousarray(np.asarray(x, dtype=np.float32))
    inh = np.ascontiguousarray(np.asarray(inhibitory_input, dtype=np.float32))
    br = np.ascontiguousarray(np.asarray(branch_input, dtype=np.float32))
    pre_w_exc = np.asarray(pre_w_exc, dtype=np.float32)
    pre_w_inh = np.asarray(pre_w_inh, dtype=np.float32)
    presigmoid_Vth = np.asarray(presigmoid_Vth, dtype=np.float32)
    log_alpha_max = np.asarray(log_alpha_max, dtype=np.float32)

    # --- replicated operands -------------------------------------------------
    # wte[p, ob, k, o] = S * W_exc[ob*P + o, k*P + p], quantized to fp8-e4m3
    we_t = (_pruned_dense_T(pre_w_exc, K_EXC) * S).astype(F8)  # [EXC_IN, OUT]
    wi_t = (_pruned_dense_T(pre_w_inh, K_INH) * S).astype(F8)  # [INH_IN, OUT]
    wte = np.ascontiguousarray(
        we_t.reshape(KE, P, OB, P).transpose(1, 2, 0, 3))
    wti = np.ascontiguousarray(
        wi_t.reshape(KI, P, OB, P).transpose(1, 2, 0, 3))

    cond = w_block.sum(axis=1, dtype=np.float32)              # [OUT]
    vth = (1.0 / (1.0 + np.exp(-presigmoid_Vth.astype(np.float64)))).astype(np.float32)
    sa = np.sqrt(np.exp(log_alpha_max.astype(np.float32)))
    cst = np.zeros((P, _C_COLS), dtype=np.float32)
    cst[:, _C_CP1:_C_CP1 + OB] = (S * (1.0 + cond)).reshape(OB, P).T
    cst[:, _C_VTHN:_C_VTHN + OB] = (-vth).reshape(OB, P).T
    cst[:, _C_SA:_C_SA + OB] = sa.reshape(OB, P).T
    cst[:, _C_WB:] = w_block.reshape(OB, P, BF).transpose(1, 0, 2).reshape(P, OB * BF)

    # --- per-core shards -----------------------------------------------------
    in_maps = []
    for c in range(NCORES):
        s = slice(c * BC, (c + 1) * BC)
        # xt[nb, p, k, b] = x[c*BC + nb*BSUB + b, k*P + p]
        xt = np.ascontiguousarray(
            x[s].astype(F8).reshape(NB, BSUB, KE, P).transpose(0, 3, 2, 1))
        iht = np.ascontiguousarray(
            inh[s].astype(F8).reshape(NB, BSUB, KI, P).transpose(0, 3, 2, 1))
        # brt[nb, ob, o, b, f] = S * branch[c*BC + nb*BSUB + b, (ob*P + o)*BF + f]
        brt = np.ascontiguousarray(
            (br[s] * S).astype(np.float16).reshape(NB, BSUB, OB, P, BF).transpose(0, 2, 3, 1, 4))
        in_maps.append({"wte": wte, "wti": wti, "cst": cst,
                        "xt": xt, "iht": iht, "brt": brt})

    res = run_bass_kernel_spmd(nc, in_maps, list(range(NCORES)), trace=TRACE)
    _CACHE["last"] = res

    out = np.empty((B, OUT), dtype=np.float32)
    for c in range(NCORES):
        # outt[ob, o, nb, b] -> out[c*BC + nb*BSUB + b, ob*P + o]
        ot = res.results[c]["outt"].astype(np.float32)
        out[c * BC:(c + 1) * BC] = ot.transpose(2, 3, 0, 1).reshape(BC, OUT)
    return out
